# revision 2
# baseline (speedup 1.0000x reference)
"""Radius-count kernel (torch.cdist + threshold + sum) for Trainium2, 8 cores.

counts[n] = #{ m : ||padding[m] - pointcloud[n]|| <= 0.5 }

Strategy
--------
d^2(n,m) <= 0.25  <=>  q(n,m) = 0.25 - |a_m|^2 - |b_n|^2 + 2 a_m.b_n >= 0.

q is a bilinear form, so each (n-tile, m-chunk) block of q is one small-K
matmul on the PE array.  To get fp32-grade accuracy at bf16 matmul speed,
every fp32 operand is decomposed exactly into 3 bf16 pieces (8 mantissa
bits each, power-of-two scales), and the matmul contracts over all piece
cross-products except the negligible lo*lo term: K = 30 rows.  Every
product of two pieces is exact in fp32, so the only error vs. the jax
reference is fp32 accumulation-order rounding (~1e-7 relative on d^2).

Each core handles 25000/8 = 3125 padding points (padded to 3136 columns)
against all 20000 pointcloud points (157 tiles of 128 partitions).

PE array packing (the big on-HW win): K=30 uses <1/4 of the 128-deep
contraction.  The PE is reconfigured as four independent 32x128 row-tiles
(tile_position=(32g,0)); lhs and rhs are replicated into all four SBUF
partition quadrants and the 8 matmuls per n-tile are issued round-robin
across the four row-groups, which stream concurrently (HW-measured ~2.8x
PE throughput; the PE in this environment runs at 1.2 GHz / 1 col/cycle
and was the actual bottleneck of the epilogue-balanced baseline at 440us).

The threshold+count epilogue is split across both PSUM-reading engines,
each consuming a whole PSUM chunk in ONE instruction (fused accumulate):
 - ScalarE: activation(Sign) with free-axis accumulation, 768 cols/half
 - VectorE: tensor_scalar(is_ge 0)+add reduction, 800 cols/half
HW-calibrated costs: ACT ~330ns/op + 0.833ns/col, DVE ~123ns/op +
1.042ns/col -> balanced split is (768, 800) per half-tile, 4 ops per
n-tile (PSUM capacity forces 4 chunks of ~784 fp32).  Engine-only floor
measured 303us/core; this kernel measures ~315us/core (repeat-delta).
"""

import numpy as np
import ml_dtypes

import os

N = 20000
M = 25000
NCORES = 8
NT = 157                 # n-tiles of 128 -> 20096 columns
NPAD = NT * 128
MS = M // NCORES         # 3125 padding points per core
# m-columns per PSUM chunk: CA via ScalarE(Sign), CD via VectorE(is_ge).
# Each chunk fits 2 PSUM banks; 2 chunks of each per n-tile.
CA = int(os.environ.get("KRN_CA", "768"))
CD = int(os.environ.get("KRN_CD", "800"))
REPEAT = int(os.environ.get("KRN_REPEAT", "1"))  # timing-only: loop body R times
MPAD = 2 * (CA + CD)     # 3136 >= 3125
K = 30                   # contraction rows
NG = 4                   # PE row-groups (32-row tiles)
ACT_COLS = 2 * CA        # m-columns counted via Sign (+-1) per core

_BF = ml_dtypes.bfloat16
_PROGRAMS = {}           # repeat -> cached compiled Bass program
LAST_RESULTS = None      # BassKernelResults of the most recent run


def _split3(x):
    """Exact 3-way bf16 decomposition of fp32 data: x == p0+p1+p2 (up to
    ~2^-25 relative from a possible carry in the last piece)."""
    x = np.asarray(x, np.float32)
    p0 = x.astype(_BF).astype(np.float32)
    r = (x - p0).astype(np.float32)
    p1 = r.astype(_BF).astype(np.float32)
    r2 = (r - p1).astype(np.float32)
    p2 = r2.astype(_BF).astype(np.float32)
    return p0, p1, p2


def _norm2(p):
    """fp32 row norms with the same op order as jnp.sum(p*p, axis=1)."""
    pp = (p * p).astype(np.float32)
    return ((pp[:, 0] + pp[:, 1]) + pp[:, 2]).astype(np.float32)


def _row_plan(B, nb, one_l, A, s, one_r):
    """The K=30 contraction rows, smallest magnitude first (PSUM partial sums
    accumulate in row order; adding small terms first minimizes rounding)."""
    rows = []
    for c in range(3):
        rows.append((B[c][1], A[c][2]))
    for c in range(3):
        rows.append((B[c][2], A[c][1]))
    for c in range(3):
        rows.append((B[c][1], A[c][1]))
    for c in range(3):
        rows.append((B[c][0], A[c][2]))
    for c in range(3):
        rows.append((B[c][2], A[c][0]))
    rows.append((nb[2], one_r))
    rows.append((one_l, s[2]))
    for c in range(3):
        rows.append((B[c][0], A[c][1]))
    for c in range(3):
        rows.append((B[c][1], A[c][0]))
    rows.append((nb[1], one_r))
    rows.append((one_l, s[1]))
    for c in range(3):
        rows.append((B[c][0], A[c][0]))
    rows.append((nb[0], one_r))
    rows.append((one_l, s[0]))
    assert len(rows) == K
    return rows


def _rep4(x):
    """Replicate [K, W] rows into all four 32-partition SBUF quadrants."""
    out = np.zeros((128, x.shape[1]), x.dtype)
    for g in range(NG):
        out[32 * g:32 * g + K] = x
    return out


def _build_operands(pointcloud, padding_shard):
    """lhs_t [128, NPAD] bf16 (pointcloud side, 4 quadrant replicas),
    rhs [128, MPAD] bf16 (padding side, 4 quadrant replicas)."""
    return _build_lhs(pointcloud), _build_rhs(padding_shard)


def _build_lhs(pointcloud):
    b = np.asarray(pointcloud, np.float32)
    nb_full = -_norm2(b)
    B = []
    for c in range(3):
        p0, p1, p2 = _split3(b[:, c])
        B.append((2.0 * p0, 2.0 * p1, 2.0 * p2))  # exact in bf16
    nb = _split3(nb_full)
    one_l = np.ones(b.shape[0], np.float32)
    zero_r = (np.zeros(1, np.float32),) * 3
    rows = _row_plan(B, nb, one_l, [zero_r] * 3, zero_r, np.zeros(1, np.float32))
    lhs = np.zeros((K, NPAD), np.float32)
    nv = b.shape[0]
    for k, (lrow, _) in enumerate(rows):
        lhs[k, :nv] = lrow
    return _rep4(lhs.astype(_BF))


def _build_rhs(padding_shard):
    a = np.asarray(padding_shard, np.float32)
    s_full = (np.float32(0.25) - _norm2(a)).astype(np.float32)
    A = []
    for c in range(3):
        A.append(_split3(a[:, c]))
    s = _split3(s_full)
    one_r = np.ones(a.shape[0], np.float32)
    zero_l = (np.zeros(1, np.float32),) * 3
    rows = _row_plan([zero_l] * 3, zero_l, np.zeros(1, np.float32), A, s, one_r)
    rhs = np.zeros((K, MPAD), np.float32)
    mv = a.shape[0]
    for k, (_, rrow) in enumerate(rows):
        rhs[k, :mv] = rrow
    # Padded m columns: q = -1 (never counted).  Row K-1 is (one_l, s[0]).
    rhs[:, mv:] = 0.0
    rhs[K - 1, mv:] = -1.0
    return _rep4(rhs.astype(_BF))


def _get_program(repeat=None):
    if repeat is None:
        repeat = REPEAT
    if repeat in _PROGRAMS:
        return _PROGRAMS[repeat]

    import concourse.bacc as bacc
    import concourse.mybir as mybir
    import concourse.tile as tile

    nc = bacc.Bacc("TRN2", target_bir_lowering=False, debug=False,
                   enable_asserts=False, num_devices=NCORES)
    f32 = mybir.dt.float32
    bf16 = mybir.dt.bfloat16
    lhs_d = nc.dram_tensor("lhs_t", [128, NPAD], bf16, kind="ExternalInput").ap()
    rhs_d = nc.dram_tensor("rhs", [128, MPAD], bf16, kind="ExternalInput").ap()
    act_d = nc.dram_tensor("actsum", [128, 2 * NT], f32, kind="ExternalOutput").ap()
    dve_d = nc.dram_tensor("dvesum", [128, 2 * NT], f32, kind="ExternalOutput").ap()

    with tile.TileContext(nc) as tc:
        with tc.tile_pool(name="const", bufs=1) as cpool, \
             tc.tile_pool(name="psA", bufs=2, space="PSUM") as psA, \
             tc.tile_pool(name="psB", bufs=2, space="PSUM") as psB, \
             tc.tile_pool(name="scr", bufs=3) as scr, \
             tc.tile_pool(name="accp", bufs=1) as accp:
            lhs_sb = cpool.tile([128, NPAD], bf16)
            nc.sync.dma_start(out=lhs_sb, in_=lhs_d)
            rhs_sb = cpool.tile([128, MPAD], bf16)
            nc.sync.dma_start(out=rhs_sb, in_=rhs_d)
            bias_sb = cpool.tile([128, 1], f32)
            nc.vector.memset(bias_sb, 1e-30)

            act_sb = accp.tile([128, 2 * NT], f32)
            dve_sb = accp.tile([128, 2 * NT], f32)

            grp = [0]

            def fill_psum(ps, t, c0, width):
                """Fill [128, width] PSUM from rhs cols [c0, c0+width), using
                the four 32-row PE tiles round-robin (concurrent streams)."""
                o = 0
                while o < width:
                    w = min(512, width - o)
                    g = grp[0] % NG
                    grp[0] += 1
                    nc.tensor.matmul(
                        ps[:, o:o + w],
                        lhs_sb[32 * g:32 * g + K, t * 128:(t + 1) * 128],
                        rhs_sb[32 * g:32 * g + K, c0 + o:c0 + o + w],
                        start=True, stop=True, tile_position=(32 * g, 0))
                    o += w

            def body():
                for t in range(NT):
                    for j in range(2):
                        base = j * (CA + CD)
                        col = 2 * t + j
                        pa = psA.tile([128, CA], f32)
                        fill_psum(pa, t, base, CA)
                        sa = scr.tile([128, CA], bf16, tag="sa")
                        nc.scalar.activation(
                            sa, pa, mybir.ActivationFunctionType.Sign,
                            bias=bias_sb, accum_out=act_sb[:, col:col + 1])
                        pb = psB.tile([128, CD], f32)
                        fill_psum(pb, t, base + CA, CD)
                        sv = scr.tile([128, CD], f32, tag="sv")
                        nc.vector.tensor_scalar(
                            sv, pb, 0.0, 0.0,
                            op0=mybir.AluOpType.is_ge, op1=mybir.AluOpType.add,
                            accum_out=dve_sb[:, col:col + 1])

            if repeat > 1:
                with tc.For_i(0, repeat, 1):
                    body()
            else:
                body()

            nc.sync.dma_start(out=act_d, in_=act_sb)
            nc.sync.dma_start(out=dve_d, in_=dve_sb)
    nc.compile()
    _PROGRAMS[repeat] = nc
    return nc


def kernel(pointcloud, pointcloud_padding):
    global LAST_RESULTS
    from concourse.bass_utils import run_bass_kernel_spmd

    pc = np.asarray(pointcloud, np.float32)
    pad = np.asarray(pointcloud_padding, np.float32)

    lhs = _build_lhs(pc)
    in_maps = [{"lhs_t": lhs, "rhs": _build_rhs(pad[i * MS:(i + 1) * MS])}
               for i in range(NCORES)]

    nc = _get_program()
    res = run_bass_kernel_spmd(nc, in_maps, core_ids=list(range(NCORES)))
    LAST_RESULTS = res

    total = np.zeros((128, NT), np.float32)
    for i in range(NCORES):
        A = res.results[i]["actsum"]
        D = res.results[i]["dvesum"]
        # Sign sums S over ACT_COLS valid +-1 entries: count = (S+ACT_COLS)/2
        total += (A[:, 0::2] + A[:, 1::2] + np.float32(ACT_COLS)) * np.float32(0.5)
        total += D[:, 0::2] + D[:, 1::2]
    counts = total.T.reshape(-1)[:N]
    return np.rint(counts).astype(np.int32).reshape(N, 1)


# revision 4
# speedup vs baseline: 1.7856x; 1.7856x over previous
"""Radius-count kernel (torch.cdist + threshold + sum) for Trainium2, 8 cores.

counts[n] = #{ m : ||padding[m] - pointcloud[n]|| <= 0.5 }

Strategy
--------
d^2(n,m) <= 0.25  <=>  q(n,m) = 0.25 - |a_m|^2 - |b_n|^2 + 2 a_m.b_n >= 0.

q is a bilinear form, so each (n-tile, m-chunk) block of q is one small-K
matmul on the PE array.  Every fp32 operand is decomposed exactly into 3
bf16 pieces (8 mantissa bits each), and the matmul contracts over all piece
cross-products except the negligible lo*lo term: K = 30 rows.  The only
error vs. the jax reference is fp32 accumulation-order rounding.

Sharding: M is split round-robin over the x-sorted padding (core c gets
sorted indices c::8), so every core sees an yearly identical x-distribution
of 3125 points.  All 20000 pointcloud points are processed by every core.

Three HW-measured optimizations over the naive balanced kernel (440us):

1. PE row-packing: K=30 uses <1/4 of the PE's 128-deep contraction, and in
   this environment the PE runs at 1.2 GHz, 1 bf16 col/cycle (it was the
   real bottleneck; the epilogue engines idle ~30%).  The PE is driven as
   four independent 32x128 row-tiles (tile_position=(32g,0)) with lhs/rhs
   replicated into all four SBUF partition quadrants; the matmuls of a tile
   are issued round-robin across row-groups and stream concurrently
   (HW-measured ~2.8x PE throughput).

2. Exact x-window pruning: pointcloud is sorted by x, so n-tile t spans an
   x-slab [xlo,xhi]; only padding columns with x in [xlo-0.5, xhi+0.5] can
   be within radius.  Padding is x-sorted per core, so that set is one
   contiguous column window, computed exactly on the host per tile (union
   over cores).  Engine + PE work shrinks ~25%; edge tiles need fewer PSUM
   chunks (fewer per-op fixed costs).  This is exact, not approximate.

3. Engine balance from HW-calibrated costs (ScalarE Sign+accum ~330ns/op +
   0.833ns/col; VectorE is_ge+accum ~123ns/op + 1.042ns/col): each tile's
   window is split into <=1024-col PSUM chunks assigned to the engines in
   the measured-rate ratio.

The threshold+count epilogue consumes each PSUM chunk in ONE instruction:
 - ScalarE: activation(Sign) with fused free-axis accumulation (sum of +-1)
 - VectorE: tensor_scalar(is_ge 0) with fused add-reduction (sum of 0/1)
Per-core partials return as [128, nops] f32 tensors; the host combines.
"""

import numpy as np
import ml_dtypes

import os

N = 20000
M = 25000
NCORES = 8
NT = 157                 # n-tiles of 128 -> 20096 columns
NPAD = NT * 128
MS = M // NCORES         # 3125 padding points per core (round-robin shard)
REPEAT = int(os.environ.get("KRN_REPEAT", "1"))  # timing-only: loop body R times
MPAD = 3136              # rhs columns per core (3125 real + 11 guard)
K = 30                   # contraction rows
NG = 4                   # PE row-groups (32-row tiles)
CHUNK = 1024             # max engine-op columns (2 PSUM banks of fp32)
# HW-calibrated engine costs (ns/op fixed, ns/col):
_FA, _RA = 330.0, 1.0 / 1.2   # ScalarE Sign+accum
_FD, _RD = 123.0, 1.0 / 0.96  # VectorE is_ge+accum

_BF = ml_dtypes.bfloat16
_PROGRAMS = {}           # (repeat, plan_key) -> compiled Bass program
_PLAN = None             # windows/chunk plan computed from the inputs
LAST_RESULTS = None


def _split3(x):
    """Exact 3-way bf16 decomposition of fp32 data: x == p0+p1+p2."""
    x = np.asarray(x, np.float32)
    p0 = x.astype(_BF).astype(np.float32)
    r = (x - p0).astype(np.float32)
    p1 = r.astype(_BF).astype(np.float32)
    r2 = (r - p1).astype(np.float32)
    p2 = r2.astype(_BF).astype(np.float32)
    return p0, p1, p2


def _norm2(p):
    pp = (p * p).astype(np.float32)
    return ((pp[:, 0] + pp[:, 1]) + pp[:, 2]).astype(np.float32)


def _row_plan(B, nb, one_l, A, s, one_r):
    """The K=30 contraction rows, smallest magnitude first."""
    rows = []
    for c in range(3):
        rows.append((B[c][1], A[c][2]))
    for c in range(3):
        rows.append((B[c][2], A[c][1]))
    for c in range(3):
        rows.append((B[c][1], A[c][1]))
    for c in range(3):
        rows.append((B[c][0], A[c][2]))
    for c in range(3):
        rows.append((B[c][2], A[c][0]))
    rows.append((nb[2], one_r))
    rows.append((one_l, s[2]))
    for c in range(3):
        rows.append((B[c][0], A[c][1]))
    for c in range(3):
        rows.append((B[c][1], A[c][0]))
    rows.append((nb[1], one_r))
    rows.append((one_l, s[1]))
    for c in range(3):
        rows.append((B[c][0], A[c][0]))
    rows.append((nb[0], one_r))
    rows.append((one_l, s[0]))
    assert len(rows) == K
    return rows


def _rep4(x):
    """Replicate [K, W] rows into all four 32-partition SBUF quadrants."""
    out = np.zeros((128, x.shape[1]), x.dtype)
    for g in range(NG):
        out[32 * g:32 * g + K] = x
    return out


def _plan_tiles(pc_sorted_x, shard_xs):
    """Per-tile m-window + chunk/engine split.

    pc_sorted_x: [NPAD] x of the sorted (and padded) pointcloud
    shard_xs: list of NCORES sorted x arrays (one per core shard)
    Returns list of per-tile dicts and accumulator column counts.
    """
    tiles = []
    na = nd = 0
    for t in range(NT):
        xlo = float(pc_sorted_x[t * 128])
        xhi = float(pc_sorted_x[t * 128 + 127])
        start = min(int(np.searchsorted(xs, xlo - 0.5 - 1e-6, "left"))
                    for xs in shard_xs)
        end = max(int(np.searchsorted(xs, xhi + 0.5 + 1e-6, "right"))
                  for xs in shard_xs)
        end = min(end, MS)
        w = end - start
        assert w > 0
        # choose per-engine op counts (ka ACT chunks, kd DVE chunks) and the
        # balanced column split minimizing max engine time for this tile
        best = None
        for ka in range(0, 5):
            for kd in range(0, 5):
                if (ka + kd) * CHUNK < w or ka + kd == 0:
                    continue
                lo = max(0, w - kd * CHUNK)
                hi = min(w, ka * CHUNK)
                if lo > hi:
                    continue
                st = (_RD * w + kd * _FD - ka * _FA) / (_RA + _RD)
                st = int(round(min(max(st, lo), hi)))
                ta = ka * _FA + _RA * st
                td = kd * _FD + _RD * (w - st)
                cost = max(ta, td) + 0.001 * (ka + kd)
                if best is None or cost < best[0]:
                    best = (cost, ka, kd, st)
        _, ka, kd, s = best

        def split(total, k):
            if k == 0 or total <= 0:
                return []
            q, r = divmod(total, k)
            return [q + (1 if i < r else 0) for i in range(k) if q + (i < r)]

        a_chunks = split(s, ka)
        d_chunks = split(w - s, kd)
        tiles.append(dict(start=start, w=w, a=a_chunks, d=d_chunks,
                          acol=na, dcol=nd))
        na += len(a_chunks)
        nd += len(d_chunks)
    return tiles, na, nd


def prepare(pointcloud, pointcloud_padding):
    """Sort/shard inputs, compute windows+chunk plan, build operands.

    Returns (plan, lhs, rhs_list) and caches plan in _PLAN.
    """
    global _PLAN
    pc = np.asarray(pointcloud, np.float32)
    pad = np.asarray(pointcloud_padding, np.float32)

    perm_n = np.argsort(pc[:, 0], kind="stable")
    pc_s = pc[perm_n]
    # pad n to NPAD with copies of the last point (counts trimmed later)
    pc_pad = np.concatenate([pc_s, np.repeat(pc_s[-1:], NPAD - N, axis=0)], 0)

    perm_m = np.argsort(pad[:, 0], kind="stable")
    pad_s = pad[perm_m]
    shards = [pad_s[c::NCORES] for c in range(NCORES)]  # each sorted in x
    shard_xs = [np.asarray(s[:, 0], np.float64) for s in shards]

    tiles, na, nd = _plan_tiles(np.asarray(pc_pad[:, 0], np.float64), shard_xs)
    plan_key = hash((na, nd, tuple((t["start"], tuple(t["a"]), tuple(t["d"]))
                                   for t in tiles)))
    _PLAN = dict(tiles=tiles, na=na, nd=nd, key=plan_key, perm_n=perm_n)

    lhs = _build_lhs(pc_pad)
    rhs_list = [_build_rhs(s) for s in shards]
    return _PLAN, lhs, rhs_list


def _build_lhs(pc_pad_sorted):
    b = np.asarray(pc_pad_sorted, np.float32)
    assert b.shape[0] == NPAD
    nb_full = -_norm2(b)
    B = []
    for c in range(3):
        p0, p1, p2 = _split3(b[:, c])
        B.append((2.0 * p0, 2.0 * p1, 2.0 * p2))  # exact in bf16
    nb = _split3(nb_full)
    one_l = np.ones(b.shape[0], np.float32)
    zero_r = (np.zeros(1, np.float32),) * 3
    rows = _row_plan(B, nb, one_l, [zero_r] * 3, zero_r, np.zeros(1, np.float32))
    lhs = np.zeros((K, NPAD), np.float32)
    for k, (lrow, _) in enumerate(rows):
        lhs[k, :] = lrow
    return _rep4(lhs.astype(_BF))


def _build_rhs(padding_shard):
    a = np.asarray(padding_shard, np.float32)
    s_full = (np.float32(0.25) - _norm2(a)).astype(np.float32)
    A = []
    for c in range(3):
        A.append(_split3(a[:, c]))
    s = _split3(s_full)
    one_r = np.ones(a.shape[0], np.float32)
    zero_l = (np.zeros(1, np.float32),) * 3
    rows = _row_plan([zero_l] * 3, zero_l, np.zeros(1, np.float32), A, s, one_r)
    rhs = np.zeros((K, MPAD), np.float32)
    mv = a.shape[0]
    for k, (_, rrow) in enumerate(rows):
        rhs[k, :mv] = rrow
    # guard columns: q = -1 (never counted)
    rhs[:, mv:] = 0.0
    rhs[K - 1, mv:] = -1.0
    return _rep4(rhs.astype(_BF))


def _get_program(repeat=None):
    if repeat is None:
        repeat = REPEAT
    assert _PLAN is not None, "call prepare() first"
    tiles, na, nd = _PLAN["tiles"], _PLAN["na"], _PLAN["nd"]
    key = (repeat, _PLAN["key"])
    if key in _PROGRAMS:
        return _PROGRAMS[key]

    import concourse.bacc as bacc
    import concourse.mybir as mybir
    import concourse.tile as tile

    nc = bacc.Bacc("TRN2", target_bir_lowering=False, debug=False,
                   enable_asserts=False, num_devices=NCORES)
    f32 = mybir.dt.float32
    bf16 = mybir.dt.bfloat16
    lhs_d = nc.dram_tensor("lhs_t", [128, NPAD], bf16, kind="ExternalInput").ap()
    rhs_d = nc.dram_tensor("rhs", [128, MPAD], bf16, kind="ExternalInput").ap()
    act_d = nc.dram_tensor("actsum", [128, max(na, 1)], f32,
                           kind="ExternalOutput").ap()
    dve_d = nc.dram_tensor("dvesum", [128, max(nd, 1)], f32,
                           kind="ExternalOutput").ap()

    with tile.TileContext(nc) as tc:
        with tc.tile_pool(name="const", bufs=1) as cpool, \
             tc.tile_pool(name="psA", bufs=2, space="PSUM") as psA, \
             tc.tile_pool(name="psB", bufs=2, space="PSUM") as psB, \
             tc.tile_pool(name="scr", bufs=3) as scr, \
             tc.tile_pool(name="accp", bufs=1) as accp:
            lhs_sb = cpool.tile([128, NPAD], bf16)
            nc.sync.dma_start(out=lhs_sb, in_=lhs_d)
            rhs_sb = cpool.tile([128, MPAD], bf16)
            nc.sync.dma_start(out=rhs_sb, in_=rhs_d)
            bias_sb = cpool.tile([128, 1], f32)
            nc.vector.memset(bias_sb, 1e-30)

            act_sb = accp.tile([128, max(na, 1)], f32)
            dve_sb = accp.tile([128, max(nd, 1)], f32)

            grp = [0]

            def fill_psum(ps, t, c0, width):
                """Fill [128, width] PSUM from rhs cols [c0, c0+width), using
                the four 32-row PE tiles round-robin (concurrent streams)."""
                o = 0
                while o < width:
                    w = min(512, width - o)
                    g = grp[0] % NG
                    grp[0] += 1
                    nc.tensor.matmul(
                        ps[:, o:o + w],
                        lhs_sb[32 * g:32 * g + K, t * 128:(t + 1) * 128],
                        rhs_sb[32 * g:32 * g + K, c0 + o:c0 + o + w],
                        start=True, stop=True, tile_position=(32 * g, 0))
                    o += w

            def body():
                for t, ti in enumerate(tiles):
                    c0 = ti["start"]
                    ops = ([("a", w) for w in ti["a"]] +
                           [("d", w) for w in ti["d"]])
                    ia = id_ = 0
                    for kind, w in ops:
                        if kind == "a":
                            pa = psA.tile([128, CHUNK], f32)
                            fill_psum(pa, t, c0, w)
                            sa = scr.tile([128, CHUNK], bf16, tag="sa")
                            nc.scalar.activation(
                                sa[:, :w], pa[:, :w],
                                mybir.ActivationFunctionType.Sign,
                                bias=bias_sb,
                                accum_out=act_sb[:, ti["acol"] + ia:
                                                 ti["acol"] + ia + 1])
                            ia += 1
                        else:
                            pb = psB.tile([128, CHUNK], f32)
                            fill_psum(pb, t, c0, w)
                            sv = scr.tile([128, CHUNK], f32, tag="sv")
                            nc.vector.tensor_scalar(
                                sv[:, :w], pb[:, :w], 0.0, 0.0,
                                op0=mybir.AluOpType.is_ge,
                                op1=mybir.AluOpType.add,
                                accum_out=dve_sb[:, ti["dcol"] + id_:
                                                 ti["dcol"] + id_ + 1])
                            id_ += 1
                        c0 += w

            if repeat > 1:
                with tc.For_i(0, repeat, 1):
                    body()
            else:
                body()

            nc.sync.dma_start(out=act_d, in_=act_sb)
            nc.sync.dma_start(out=dve_d, in_=dve_sb)
    nc.compile()
    _PROGRAMS[key] = nc
    return nc


def kernel(pointcloud, pointcloud_padding):
    global LAST_RESULTS
    from concourse.bass_utils import run_bass_kernel_spmd

    plan, lhs, rhs_list = prepare(pointcloud, pointcloud_padding)
    in_maps = [{"lhs_t": lhs, "rhs": rhs_list[i]} for i in range(NCORES)]

    nc = _get_program()
    res = run_bass_kernel_spmd(nc, in_maps, core_ids=list(range(NCORES)))
    LAST_RESULTS = res

    tiles = plan["tiles"]
    total = np.zeros((128, NT), np.float32)
    for i in range(NCORES):
        A = res.results[i]["actsum"]
        D = res.results[i]["dvesum"]
        for t, ti in enumerate(tiles):
            for ia, w in enumerate(ti["a"]):
                # Sign sums S over w +-1 entries: count = (S + w) / 2
                total[:, t] += (A[:, ti["acol"] + ia] + np.float32(w)) \
                    * np.float32(0.5)
            for id_, w in enumerate(ti["d"]):
                total[:, t] += D[:, ti["dcol"] + id_]
    counts_sorted = total.T.reshape(-1)[:N]
    counts = np.empty(N, np.float32)
    counts[plan["perm_n"]] = counts_sorted
    return np.rint(counts).astype(np.int32).reshape(N, 1)


# revision 5
# speedup vs baseline: 2.0164x; 1.1293x over previous
"""Radius-count kernel (torch.cdist + threshold + sum) for Trainium2, 8 cores.

counts[n] = #{ m : ||padding[m] - pointcloud[n]|| <= 0.5 }

Strategy
--------
d^2(n,m) <= 0.25  <=>  q(n,m) = 0.25 - |a_m|^2 - |b_n|^2 + 2 a_m.b_n >= 0.

q is a bilinear form, so each (n-tile, m-chunk) block of q is one small-K
matmul on the PE array.  Every fp32 operand is decomposed exactly into 3
bf16 pieces (8 mantissa bits each), and the matmul contracts over all piece
cross-products except the negligible lo*lo term: K = 30 rows.  The only
error vs. the jax reference is fp32 accumulation-order rounding.

Sharding: M is split round-robin over the x-sorted padding (core c gets
sorted indices c::8), so every core sees an yearly identical x-distribution
of 3125 points.  All 20000 pointcloud points are processed by every core.

Three HW-measured optimizations over the naive balanced kernel (440us):

1. PE row-packing: K=30 uses <1/4 of the PE's 128-deep contraction, and in
   this environment the PE runs at 1.2 GHz, 1 bf16 col/cycle (it was the
   real bottleneck; the epilogue engines idle ~30%).  The PE is driven as
   four independent 32x128 row-tiles (tile_position=(32g,0)) with lhs/rhs
   replicated into all four SBUF partition quadrants; the matmuls of a tile
   are issued round-robin across row-groups and stream concurrently
   (HW-measured ~2.8x PE throughput).

2. Exact x-window pruning: pointcloud is sorted by x, so n-tile t spans an
   x-slab [xlo,xhi]; only padding columns with x in [xlo-0.5, xhi+0.5] can
   be within radius.  Padding is x-sorted per core, so that set is one
   contiguous column window, computed exactly on the host per tile (union
   over cores).  Engine + PE work shrinks ~25%; edge tiles need fewer PSUM
   chunks (fewer per-op fixed costs).  This is exact, not approximate.

3. Engine balance from HW-calibrated costs (ScalarE Sign+accum ~330ns/op +
   0.833ns/col; VectorE is_ge+accum ~123ns/op + 1.042ns/col): each tile's
   window is split into <=1024-col PSUM chunks assigned to the engines in
   the measured-rate ratio.

The threshold+count epilogue consumes each PSUM chunk in ONE instruction:
 - ScalarE: activation(Sign) with fused free-axis accumulation (sum of +-1)
 - VectorE: tensor_scalar(is_ge 0) with fused add-reduction (sum of 0/1)
Per-core partials return as [128, nops] f32 tensors; the host combines.
"""

import numpy as np
import ml_dtypes

import os

N = 20000
M = 25000
NCORES = 8
NT = 157                 # n-tiles of 128 -> 20096 columns
NPAD = NT * 128
MS = M // NCORES         # 3125 padding points per core (round-robin shard)
REPEAT = int(os.environ.get("KRN_REPEAT", "1"))  # timing-only: loop body R times
MPAD = 3136              # rhs columns per core (3125 real + 11 guard)
K = 30                   # contraction rows
NG = 4                   # PE row-groups (32-row tiles)
CHUNK = 1024             # max engine-op columns (2 PSUM banks of fp32)
# HW-calibrated engine costs (ns/op fixed, ns/col):
_FA, _RA = 330.0, 1.0 / 1.2   # ScalarE Sign+accum
_FD, _RD = 123.0, 1.0 / 0.96  # VectorE is_ge+accum

_BF = ml_dtypes.bfloat16
_PROGRAMS = {}           # (repeat, plan_key) -> compiled Bass program
_PLAN = None             # windows/chunk plan computed from the inputs
LAST_RESULTS = None


def _split3(x):
    """Exact 3-way bf16 decomposition of fp32 data: x == p0+p1+p2."""
    x = np.asarray(x, np.float32)
    p0 = x.astype(_BF).astype(np.float32)
    r = (x - p0).astype(np.float32)
    p1 = r.astype(_BF).astype(np.float32)
    r2 = (r - p1).astype(np.float32)
    p2 = r2.astype(_BF).astype(np.float32)
    return p0, p1, p2


def _norm2(p):
    pp = (p * p).astype(np.float32)
    return ((pp[:, 0] + pp[:, 1]) + pp[:, 2]).astype(np.float32)


def _row_plan(B, nb, one_l, A, s, one_r):
    """The K=30 contraction rows, smallest magnitude first."""
    rows = []
    for c in range(3):
        rows.append((B[c][1], A[c][2]))
    for c in range(3):
        rows.append((B[c][2], A[c][1]))
    for c in range(3):
        rows.append((B[c][1], A[c][1]))
    for c in range(3):
        rows.append((B[c][0], A[c][2]))
    for c in range(3):
        rows.append((B[c][2], A[c][0]))
    rows.append((nb[2], one_r))
    rows.append((one_l, s[2]))
    for c in range(3):
        rows.append((B[c][0], A[c][1]))
    for c in range(3):
        rows.append((B[c][1], A[c][0]))
    rows.append((nb[1], one_r))
    rows.append((one_l, s[1]))
    for c in range(3):
        rows.append((B[c][0], A[c][0]))
    rows.append((nb[0], one_r))
    rows.append((one_l, s[0]))
    assert len(rows) == K
    return rows


def _rep4(x):
    """Replicate [K, W] rows into all four 32-partition SBUF quadrants."""
    out = np.zeros((128, x.shape[1]), x.dtype)
    for g in range(NG):
        out[32 * g:32 * g + K] = x
    return out


def _plan_tiles(pc_sorted_x, shard_xs):
    """Per-tile m-window + chunk/engine split.

    pc_sorted_x: [NPAD] x of the sorted (and padded) pointcloud
    shard_xs: list of NCORES sorted x arrays (one per core shard)
    Returns list of per-tile dicts and accumulator column counts.
    """
    tiles = []
    na = nd = 0
    for t in range(NT):
        xlo = float(pc_sorted_x[t * 128])
        xhi = float(pc_sorted_x[t * 128 + 127])
        start = min(int(np.searchsorted(xs, xlo - 0.5 - 1e-6, "left"))
                    for xs in shard_xs)
        end = max(int(np.searchsorted(xs, xhi + 0.5 + 1e-6, "right"))
                  for xs in shard_xs)
        end = min(end, MS)
        w = end - start
        assert w > 0
        # choose per-engine op counts (ka ACT chunks, kd DVE chunks) and the
        # balanced column split minimizing max engine time for this tile
        best = None
        for ka in range(0, 5):
            for kd in range(0, 5):
                if (ka + kd) * CHUNK < w or ka + kd == 0:
                    continue
                lo = max(0, w - kd * CHUNK)
                hi = min(w, ka * CHUNK)
                if lo > hi:
                    continue
                st = (_RD * w + kd * _FD - ka * _FA) / (_RA + _RD)
                st = int(round(min(max(st, lo), hi)))
                ta = ka * _FA + _RA * st
                td = kd * _FD + _RD * (w - st)
                cost = max(ta, td) + 0.001 * (ka + kd)
                if best is None or cost < best[0]:
                    best = (cost, ka, kd, st)
        _, ka, kd, s = best

        def split(total, k):
            if k == 0 or total <= 0:
                return []
            q, r = divmod(total, k)
            return [q + (1 if i < r else 0) for i in range(k) if q + (i < r)]

        a_chunks = split(s, ka)
        d_chunks = split(w - s, kd)
        tiles.append(dict(start=start, w=w, a=a_chunks, d=d_chunks,
                          acol=na, dcol=nd))
        na += len(a_chunks)
        nd += len(d_chunks)
    return tiles, na, nd


def prepare(pointcloud, pointcloud_padding):
    """Sort/shard inputs, compute windows+chunk plan, build operands.

    Returns (plan, lhs, rhs_list) and caches plan in _PLAN.
    """
    global _PLAN
    pc = np.asarray(pointcloud, np.float32)
    pad = np.asarray(pointcloud_padding, np.float32)

    perm_n = np.argsort(pc[:, 0], kind="stable")
    pc_s = pc[perm_n]
    # pad n to NPAD with copies of the last point (counts trimmed later)
    pc_pad = np.concatenate([pc_s, np.repeat(pc_s[-1:], NPAD - N, axis=0)], 0)

    perm_m = np.argsort(pad[:, 0], kind="stable")
    pad_s = pad[perm_m]
    shards = [pad_s[c::NCORES] for c in range(NCORES)]  # each sorted in x
    shard_xs = [np.asarray(s[:, 0], np.float64) for s in shards]

    tiles, na, nd = _plan_tiles(np.asarray(pc_pad[:, 0], np.float64), shard_xs)
    plan_key = hash((na, nd, tuple((t["start"], tuple(t["a"]), tuple(t["d"]))
                                   for t in tiles)))
    _PLAN = dict(tiles=tiles, na=na, nd=nd, key=plan_key, perm_n=perm_n)

    lhs = _build_lhs(pc_pad)
    rhs_list = [_build_rhs(s) for s in shards]
    return _PLAN, lhs, rhs_list


def _build_lhs(pc_pad_sorted):
    b = np.asarray(pc_pad_sorted, np.float32)
    assert b.shape[0] == NPAD
    nb_full = -_norm2(b)
    B = []
    for c in range(3):
        p0, p1, p2 = _split3(b[:, c])
        B.append((2.0 * p0, 2.0 * p1, 2.0 * p2))  # exact in bf16
    nb = _split3(nb_full)
    one_l = np.ones(b.shape[0], np.float32)
    zero_r = (np.zeros(1, np.float32),) * 3
    rows = _row_plan(B, nb, one_l, [zero_r] * 3, zero_r, np.zeros(1, np.float32))
    lhs = np.zeros((K, NPAD), np.float32)
    for k, (lrow, _) in enumerate(rows):
        lhs[k, :] = lrow
    return _rep4(lhs.astype(_BF))


def _build_rhs(padding_shard):
    a = np.asarray(padding_shard, np.float32)
    s_full = (np.float32(0.25) - _norm2(a)).astype(np.float32)
    A = []
    for c in range(3):
        A.append(_split3(a[:, c]))
    s = _split3(s_full)
    one_r = np.ones(a.shape[0], np.float32)
    zero_l = (np.zeros(1, np.float32),) * 3
    rows = _row_plan([zero_l] * 3, zero_l, np.zeros(1, np.float32), A, s, one_r)
    rhs = np.zeros((K, MPAD), np.float32)
    mv = a.shape[0]
    for k, (_, rrow) in enumerate(rows):
        rhs[k, :mv] = rrow
    # guard columns: q = -1 (never counted)
    rhs[:, mv:] = 0.0
    rhs[K - 1, mv:] = -1.0
    return _rep4(rhs.astype(_BF))


def _get_program(repeat=None):
    if repeat is None:
        repeat = REPEAT
    assert _PLAN is not None, "call prepare() first"
    tiles, na, nd = _PLAN["tiles"], _PLAN["na"], _PLAN["nd"]
    key = (repeat, _PLAN["key"])
    if key in _PROGRAMS:
        return _PROGRAMS[key]

    import concourse.bacc as bacc
    import concourse.mybir as mybir
    import concourse.tile as tile

    nc = bacc.Bacc("TRN2", target_bir_lowering=False, debug=False,
                   enable_asserts=False, num_devices=NCORES)
    f32 = mybir.dt.float32
    bf16 = mybir.dt.bfloat16
    lhs_d = nc.dram_tensor("lhs_t", [128, NPAD], bf16, kind="ExternalInput").ap()
    rhs_d = nc.dram_tensor("rhs", [128, MPAD], bf16, kind="ExternalInput").ap()
    act_d = nc.dram_tensor("actsum", [128, max(na, 1)], f32,
                           kind="ExternalOutput").ap()
    dve_d = nc.dram_tensor("dvesum", [128, max(nd, 1)], f32,
                           kind="ExternalOutput").ap()

    with tile.TileContext(nc) as tc:
        with tc.tile_pool(name="const", bufs=1) as cpool, \
             tc.tile_pool(name="psA", bufs=2, space="PSUM") as psA, \
             tc.tile_pool(name="psB", bufs=2, space="PSUM") as psB, \
             tc.tile_pool(name="scr", bufs=3) as scr, \
             tc.tile_pool(name="accp", bufs=1) as accp:
            lhs_sb = cpool.tile([128, NPAD], bf16)
            nc.sync.dma_start(out=lhs_sb, in_=lhs_d)
            rhs_sb = cpool.tile([128, MPAD], bf16)
            nc.sync.dma_start(out=rhs_sb, in_=rhs_d)
            bias_sb = cpool.tile([128, 1], f32)
            nc.vector.memset(bias_sb, 1e-30)

            act_sb = accp.tile([128, max(na, 1)], f32)
            dve_sb = accp.tile([128, max(nd, 1)], f32)

            grp = [0]

            def fill_psum(ps, t, c0, width):
                """Fill [128, width] PSUM from rhs cols [c0, c0+width), using
                the four 32-row PE tiles round-robin (concurrent streams)."""
                o = 0
                while o < width:
                    w = min(512, width - o)
                    g = grp[0] % NG
                    grp[0] += 1
                    nc.tensor.matmul(
                        ps[:, o:o + w],
                        lhs_sb[32 * g:32 * g + K, t * 128:(t + 1) * 128],
                        rhs_sb[32 * g:32 * g + K, c0 + o:c0 + o + w],
                        start=True, stop=True, tile_position=(32 * g, 0))
                    o += w

            oporder = os.environ.get("KRN_OPORDER", "ad")

            def body():
                for t, ti in enumerate(tiles):
                    c0 = ti["start"]
                    if oporder == "inter":  # a1 d1 a2 d2 ...
                        ops = []
                        for i in range(max(len(ti["a"]), len(ti["d"]))):
                            if i < len(ti["a"]):
                                ops.append(("a", ti["a"][i]))
                            if i < len(ti["d"]):
                                ops.append(("d", ti["d"][i]))
                    elif oporder == "da":
                        ops = ([("d", w) for w in ti["d"]] +
                               [("a", w) for w in ti["a"]])
                    else:
                        ops = ([("a", w) for w in ti["a"]] +
                               [("d", w) for w in ti["d"]])
                    ia = id_ = 0
                    for kind, w in ops:
                        if kind == "a":
                            pa = psA.tile([128, CHUNK], f32)
                            fill_psum(pa, t, c0, w)
                            sa = scr.tile([128, CHUNK], bf16, tag="sa")
                            nc.scalar.activation(
                                sa[:, :w], pa[:, :w],
                                mybir.ActivationFunctionType.Sign,
                                bias=bias_sb,
                                accum_out=act_sb[:, ti["acol"] + ia:
                                                 ti["acol"] + ia + 1])
                            ia += 1
                        else:
                            pb = psB.tile([128, CHUNK], f32)
                            fill_psum(pb, t, c0, w)
                            sv = scr.tile([128, CHUNK], f32, tag="sv")
                            nc.vector.tensor_scalar(
                                sv[:, :w], pb[:, :w], 0.0, 0.0,
                                op0=mybir.AluOpType.is_ge,
                                op1=mybir.AluOpType.add,
                                accum_out=dve_sb[:, ti["dcol"] + id_:
                                                 ti["dcol"] + id_ + 1])
                            id_ += 1
                        c0 += w

            if repeat > 1:
                with tc.For_i(0, repeat, 1):
                    body()
            else:
                body()

            nc.sync.dma_start(out=act_d, in_=act_sb)
            nc.sync.dma_start(out=dve_d, in_=dve_sb)
    nc.compile()
    _PROGRAMS[key] = nc
    return nc


def kernel(pointcloud, pointcloud_padding):
    global LAST_RESULTS
    from concourse.bass_utils import run_bass_kernel_spmd

    plan, lhs, rhs_list = prepare(pointcloud, pointcloud_padding)
    in_maps = [{"lhs_t": lhs, "rhs": rhs_list[i]} for i in range(NCORES)]

    nc = _get_program()
    res = run_bass_kernel_spmd(nc, in_maps, core_ids=list(range(NCORES)))
    LAST_RESULTS = res

    tiles = plan["tiles"]
    total = np.zeros((128, NT), np.float32)
    for i in range(NCORES):
        A = res.results[i]["actsum"]
        D = res.results[i]["dvesum"]
        for t, ti in enumerate(tiles):
            for ia, w in enumerate(ti["a"]):
                # Sign sums S over w +-1 entries: count = (S + w) / 2
                total[:, t] += (A[:, ti["acol"] + ia] + np.float32(w)) \
                    * np.float32(0.5)
            for id_, w in enumerate(ti["d"]):
                total[:, t] += D[:, ti["dcol"] + id_]
    counts_sorted = total.T.reshape(-1)[:N]
    counts = np.empty(N, np.float32)
    counts[plan["perm_n"]] = counts_sorted
    return np.rint(counts).astype(np.int32).reshape(N, 1)
